# revision 46
# baseline (speedup 1.0000x reference)
"""Causal self-attention (muP scaling) for Trainium2, sharded over 8 NeuronCores.

Sharding: data-parallel over batch (B=2) x tensor-parallel over head groups
(16 heads -> 4 groups of 4). Core c handles batch c//4, head group c%4.
Each core computes q/k/v projections for its 256 features, causal attention
for its 4 heads, and a row-parallel partial of the output projection.
The host sums the 4 partials per batch element.

Host-side sharding also pre-transposes the shards (x^T and the weight slices
with the contraction dim leading) so every matmul operand DMAs straight into
its SBUF layout - no on-chip transposes at all.

Per-core pipeline (all matmuls float32r: full PE rate at free-dim >= 256):
 - q^T,k^T = W x^T with features on partitions; V in natural layout with an
   extra ones-column per head (so the softmax denominator Z falls out of the
   PV matmul for free).
 - S^T = K Q^T is computed tile-by-tile with keys on partitions, so the
   softmax tiles feed the PV matmul without any transpose. Softmax skips the
   max-subtraction (muP logits: std ~0.125, |logit| < ~1), exp runs on
   ScalarE straight out of PSUM, and causal masking is done by trimming the
   computed column window per diagonal block plus two small constant masks.
 - y^T is normalised by 1/Z via a partition-broadcast multiply, then the
   output projection produces this core's row-parallel partial.
"""

import os
import sys

for _p in ("/opt/trn_rl_repo",):
    if _p not in sys.path:
        sys.path.insert(0, _p)

import numpy as np

import concourse.bass as bass  # noqa: F401
import concourse.mybir as mybir
import concourse.tile as tile
from concourse import bacc
from concourse.bass_utils import run_bass_kernel_spmd
from concourse.masks import make_upper_triangular
from concourse.tile import ScopedClock

# ---- problem constants (hardcoded per contract) ----
B, T, C = 2, 2048, 1024
NH, DH = 16, 64
N_CORES = 8
GROUPS = 4                 # head groups (tensor parallel)
NH_LOC = NH // GROUPS      # 4 heads per core
F = NH_LOC * DH            # 256 per-core qkv features
P = 128
CC = C // P                # 8 contraction chunks over C
TQ = 512                   # Tq tile width
NJ = T // TQ               # 4 Tq tiles
NTC = T // P               # 16 T chunks of 128
FC = F // P                # 2 feature chunks
f32 = mybir.dt.float32
f32r = mybir.dt.float32r
EXP = mybir.ActivationFunctionType.Exp


def _install_drain_patch():
    """This walrus build rejects >2 sem waits on a single instruction; the
    Tile tail drain accumulates one wait per live proc. Split them into
    single-wait SP nops ahead of the drain."""
    if getattr(tile.TileContext, "_drain_patch_installed", False):
        return

    def _patched(self, tick_clock, wait_clock):
        nc = self.nc
        probe = nc.sync.nop(nofuse=True)
        wait_clock.add_sem_waits(
            probe.ins, ScopedClock({None: tick_clock.global_clock})
        )
        si = probe.ins.sync_info
        waits = list(si.on_wait) if si is not None and si.on_wait else []
        if len(waits) > 1:
            probe.ins.sync_info.on_wait = [waits[0]]
            for w in waits[1:]:
                n2 = nc.sync.nop(nofuse=True)
                n2.ins.sync_info = mybir.SyncInfo(on_wait=[w], on_update=[])
        nc.sync.drain()
        nc.all_engine_barrier()
        assert self.sems is not None
        popped = nc._tile_sem_poison_stack.pop()
        assert popped is self._sem_poison
        nc.clear_and_free_semaphores(list(self.sems.allocated().values()))
        nc.all_engine_barrier()

    tile.TileContext._drain_and_barrier = _patched
    tile.TileContext._drain_patch_installed = True


def build_module():
    """Build the per-core Bass module (uniform across all 8 cores)."""
    _install_drain_patch()
    nc = bacc.Bacc("TRN2", target_bir_lowering=False, debug=False)
    xt = nc.dram_tensor("xt", [C, T], f32r, kind="ExternalInput").ap()
    wqt = nc.dram_tensor("wqt", [C, F], f32r, kind="ExternalInput").ap()
    wkt = nc.dram_tensor("wkt", [C, F], f32r, kind="ExternalInput").ap()
    wvt = nc.dram_tensor("wvt", [C, F], f32r, kind="ExternalInput").ap()
    wpt = nc.dram_tensor("wpt", [F, C], f32r, kind="ExternalInput").ap()
    out = nc.dram_tensor("out", [T, C], f32, kind="ExternalOutput").ap()

    with tile.TileContext(nc) as tc:
        _body(tc, xt, wqt, wkt, wvt, wpt, out)
    nc.compile()
    return nc


def _body(tc, xt, wqt, wkt, wvt, wpt, out):
    from contextlib import ExitStack

    nc = tc.nc
    with ExitStack() as ctx:
        const = ctx.enter_context(tc.tile_pool(name="const", bufs=1))
        wpool = ctx.enter_context(tc.tile_pool(name="wpool", bufs=1))
        qkv = ctx.enter_context(tc.tile_pool(name="qkv", bufs=1))
        xtp = ctx.enter_context(tc.tile_pool(name="xtp", bufs=3))
        sexp = ctx.enter_context(tc.tile_pool(name="sexp", bufs=8))
        ytile = ctx.enter_context(tc.tile_pool(name="ytile", bufs=2))
        small = ctx.enter_context(tc.tile_pool(name="small", bufs=4))
        outp = ctx.enter_context(tc.tile_pool(name="outp", bufs=4))
        ps_s1 = ctx.enter_context(tc.tile_pool(name="ps_s1", bufs=3, space="PSUM"))
        ps_s = ctx.enter_context(tc.tile_pool(name="ps_s", bufs=4, space="PSUM"))
        ps_y = ctx.enter_context(tc.tile_pool(name="ps_y", bufs=1, space="PSUM"))

        # masks for the causal diagonal blocks of S^T (valid = tk <= tq):
        # umask: [128,128] upper triangular (incl diag) ones
        # m2:    [128,256] = [zeros | umask] for the d==3 trimmed window
        umask = const.tile([P, P], f32)
        make_upper_triangular(nc, umask, val=1.0, diag=True)
        m2 = const.tile([P, 2 * P], f32)
        nc.vector.memset(m2[:, 0:P], 0.0)
        nc.vector.tensor_copy(m2[:, P:2 * P], umask)

        # ---- stage 0: weights straight from HBM (already transposed) ----
        wqT = wpool.tile([P, CC, F], f32r)   # [c-part, c-chunk, feat]
        wkT = wpool.tile([P, CC, F], f32r)
        wvT = wpool.tile([P, CC, F], f32r)
        wpT = wpool.tile([P, FC, C], f32r)   # [f-part, f-chunk, out]
        # chunked loads so the first matmuls can start as soon as their
        # c-chunk lands; wq/wk/x(j=0) chunks first (they gate q^T/k^T),
        # wv after, wproj last
        xt_r = xt.rearrange("(cc p) t -> p cc t", p=P)
        xT0 = xtp.tile([P, CC, TQ], f32r, tag="xT", name="xT_0")
        wq_r = wqt.rearrange("(cc p) f -> p cc f", p=P)
        wk_r = wkt.rearrange("(cc p) f -> p cc f", p=P)
        for cc in range(CC):
            nc.sync.dma_start(out=wqT[:, cc], in_=wq_r[:, cc])
            nc.sync.dma_start(out=wkT[:, cc], in_=wk_r[:, cc])
            nc.sync.dma_start(out=xT0[:, cc], in_=xt_r[:, cc, 0:TQ])
        nc.sync.dma_start(out=wvT, in_=wvt.rearrange("(cc p) f -> p cc f", p=P))
        nc.sync.dma_start(out=wpT, in_=wpt.rearrange("(fc p) o -> p fc o", p=P))

        # ---- persistent q^T, k^T, V' ----
        qT = qkv.tile([P, FC, T], f32r)      # [feat-part, f-chunk, t]
        kT = qkv.tile([P, FC, T], f32r)
        Vp = qkv.tile([P, NTC, NH_LOC * (DH + 1)], f32r)
        # ones-column trick: col DH of each head block is 1.0 so the PV
        # matmul's last output row accumulates the softmax denominator Z
        ones16 = const.tile([P, NTC], f32)
        nc.vector.memset(ones16, 1.0)
        for h in range(NH_LOC):
            nc.vector.tensor_copy(Vp[:, :, h * (DH + 1) + DH], ones16)

        scale = 1.0 / float(DH)
        for j in range(NJ):
            # ---- stage 1: load x^T slab, then q^T/k^T/V ----
            if j == 0:
                xT = xT0
            else:
                xT = xtp.tile([P, CC, TQ], f32r, tag="xT", name=f"xT_{j}")
                for cc2 in range(CC):
                    nc.sync.dma_start(
                        out=xT[:, cc2],
                        in_=xt_r[:, cc2, j * TQ:(j + 1) * TQ],
                    )

            # fc-major order: q/k feature-chunk 0 land first so the next
            # tile's attention (heads 0/1) can start before fc1 finishes
            for fc in range(FC):
                for wT, dstT in ((wqT, qT), (wkT, kT)):
                    pq = ps_s1.tile([P, 512], f32, tag="s1")
                    for cc in range(CC):
                        nc.tensor.matmul(
                            pq,
                            lhsT=wT[:, cc, fc * P:(fc + 1) * P],
                            rhs=xT[:, cc, :],
                            start=(cc == 0),
                            stop=(cc == CC - 1),
                        )
                    nc.vector.tensor_copy(dstT[:, fc, j * TQ:(j + 1) * TQ], pq)

            for r in range(TQ // P):
                pv = ps_s1.tile([P, 512], f32, tag="s1")
                for cc in range(CC):
                    nc.tensor.matmul(
                        pv[:, :F],
                        lhsT=xT[:, cc, r * P:(r + 1) * P],
                        rhs=wvT[:, cc, :],
                        start=(cc == 0),
                        stop=(cc == CC - 1),
                    )
                nc.vector.tensor_copy(
                    Vp[:, 4 * j + r].rearrange("p (h c) -> p h c", c=DH + 1)[
                        :, :, 0:DH
                    ],
                    pv[:, :F],
                )

            # ---- stage 2: attention for this Tq tile ----
            yts = [
                ytile.tile([P, TQ], f32r, tag=f"yt{fc}", name=f"yt{fc}_{j}")
                for fc in range(FC)
            ]
            for h in range(NH_LOC):
                fc, ro = h // 2, (h % 2) * DH
                py = ps_y.tile([DH + 1, TQ], f32, tag="y")
                n_i = 4 * j + 4
                for i in range(n_i):
                    d = i - 4 * j
                    # computed column window [start, TQ): skips fully-masked
                    # leading columns, kept >=256 wide for full-rate fp32r
                    start = 0 if d <= 0 else min(128 * d, 256)
                    ps = ps_s.tile([P, 512], f32, tag="s")
                    nc.tensor.matmul(
                        ps[:, start:],
                        lhsT=kT[ro:ro + DH, fc, i * P:(i + 1) * P],
                        rhs=qT[ro:ro + DH, fc, j * TQ + start:(j + 1) * TQ],
                        start=True,
                        stop=True,
                    )
                    se = sexp.tile([P, TQ], f32r, tag="sexp")
                    nc.scalar.activation(
                        se[:, start:], ps[:, start:], EXP, scale=scale
                    )
                    if d == 3:
                        nc.vector.tensor_mul(
                            se[:, 256:512], se[:, 256:512], m2
                        )
                    elif d >= 0:
                        nc.vector.tensor_mul(
                            se[:, 128 * d:128 * (d + 1)],
                            se[:, 128 * d:128 * (d + 1)],
                            umask,
                        )
                    nc.tensor.matmul(
                        py[:, start:],
                        lhsT=Vp[:, i, h * (DH + 1):(h + 1) * (DH + 1)],
                        rhs=se[:, start:],
                        start=(i == 0),
                        stop=(i == n_i - 1),
                    )
                # normalise: yts[fc][ro:ro+DH] = py[0:DH] * (1/Z) broadcast
                rc = small.tile([1, TQ], f32, tag="recip")
                nc.vector.reciprocal(rc, py[DH:DH + 1, :])
                rb = small.tile([DH, TQ], f32, tag="rb")
                nc.gpsimd.partition_broadcast(rb, rc)
                nc.vector.tensor_mul(yts[fc][ro:ro + DH, :], py[0:DH, :], rb)

            # ---- stage 3: output projection (row-parallel partial) ----
            # on the last Tq tile ScalarE is done with exp, so split the
            # PSUM evacuations between ScalarE and VectorE and DMA each
            # 512-col half out as soon as it lands to shorten the tail
            for q in range(TQ // P):
                ob = outp.tile([P, C], f32, tag="ob")
                for n in range(C // 512):
                    po = ps_s.tile([P, 512], f32, tag="s", name=f"po_{j}_{q}_{n}")
                    for fc in range(FC):
                        nc.tensor.matmul(
                            po,
                            lhsT=yts[fc][:, q * P:(q + 1) * P],
                            rhs=wpT[:, fc, n * 512:(n + 1) * 512],
                            start=(fc == 0),
                            stop=(fc == FC - 1),
                        )
                    if j == NJ - 1 and n % 2 == 1:
                        nc.scalar.copy(ob[:, n * 512:(n + 1) * 512], po)
                    else:
                        nc.vector.tensor_copy(ob[:, n * 512:(n + 1) * 512], po)
                    if j == NJ - 1:
                        nc.sync.dma_start(
                            out=out[
                                j * TQ + q * P: j * TQ + (q + 1) * P,
                                n * 512:(n + 1) * 512,
                            ],
                            in_=ob[:, n * 512:(n + 1) * 512],
                        )
                if j < NJ - 1:
                    nc.sync.dma_start(
                        out=out[j * TQ + q * P: j * TQ + (q + 1) * P, :], in_=ob
                    )


_CACHE = {}


def shard_inputs(x, wq, wk, wv, wproj):
    in_maps = []
    for c in range(N_CORES):
        b, g = divmod(c, GROUPS)
        sl = slice(g * F, (g + 1) * F)
        in_maps.append(
            {
                "xt": np.ascontiguousarray(x[b].T, dtype=np.float32),
                "wqt": np.ascontiguousarray(wq[sl, :].T, dtype=np.float32),
                "wkt": np.ascontiguousarray(wk[sl, :].T, dtype=np.float32),
                "wvt": np.ascontiguousarray(wv[sl, :].T, dtype=np.float32),
                "wpt": np.ascontiguousarray(wproj[:, sl].T, dtype=np.float32),
            }
        )
    return in_maps


def kernel(x, wq, wk, wv, wproj):
    x = np.asarray(x, dtype=np.float32)
    wq = np.asarray(wq, dtype=np.float32)
    wk = np.asarray(wk, dtype=np.float32)
    wv = np.asarray(wv, dtype=np.float32)
    wproj = np.asarray(wproj, dtype=np.float32)

    from concourse._compat import axon_active

    if axon_active():
        # the axon NTFF-profile hook isn't available in this environment;
        # a BASS_TRACE=1 run would crash importing it, so disable tracing
        os.environ.setdefault("BASS_NEVER_TRACE", "1")

    if "nc" not in _CACHE:
        _CACHE["nc"] = build_module()
    nc = _CACHE["nc"]

    in_maps = shard_inputs(x, wq, wk, wv, wproj)
    res = run_bass_kernel_spmd(nc, in_maps, core_ids=list(range(N_CORES)))
    out = np.zeros((B, T, C), np.float32)
    for c in range(N_CORES):
        b = c // GROUPS
        out[b] += res.results[c]["out"]
    return out
